# revision 5
# baseline (speedup 1.0000x reference)
"""Trainium2 Bass kernel for CompressedInteractionNetwork (CIN) forward.

Reference computation (per sample):
  x0 = x (F=32 fields, E=16 embed), h = x
  layer i: z = outer(x0, h) over fields -> (F*G_i, E); y = relu(W_i @ z + b_i)
  layers 0,1: keep = y[:64] -> output, h = y[64:]
  layer 2: keep = y
  out = concat(keeps) summed over E -> (B, 256)

Strategy: pure data parallelism over batch (4096 -> 512 per core, 8 cores).
Per core, n = (b_local, e) is the matmul free dim (8192 total, chunks of 512).
Compute in fp16, fp32 PSUM / bias / relu / e-sum.

Software-pipelined stages: at stage s the PE runs mm0(s), mm1(s-1), mm2(s-2)
(three independent chunks), so every matmul's z operand was formed a stage
earlier and PE stays dense (full p-state). Per chunk c:
  stage c-2: xr0(c) DMA     [x rows 4t..4t+3 each x32, 4 batched DMAs]
  stage c-1: xr12(c) DMA    [rows 2t,2t+1 each x64, 2 DMAs]; z0(c)
  stage c:   mm0(c), ACT evac h1/kt0, z1(c), e-sum kt0
  stage c+1: mm1(c), ACT evac h2/kt1, z2(c) (in place on xr12), e-sum kt1
  stage c+2: mm2(c), ACT evac kt2, e-sum kt2
z = Xrep * Hrep elementwise, split DVE/Pool; Hrep via stride-0 broadcast
(layer 0: x4 static; layers 1,2: [h;h] mirror built by two ACT evacs).
All DMAs issue from SP (HWDGE); outputs accumulate in SBUF, one DMA at end.
"""
import numpy as np

B = 4096
F = 32
E = 16
O = 128
N_CORES = 8
BC = B // N_CORES          # 512 samples per core
NTOT = BC * E              # 8192 n-columns per core
NCHUNK = 512               # matmul free dim per chunk
NCHUNKS = NTOT // NCHUNK   # 16
BCH = NCHUNK // E          # 32 samples per chunk
TS = (8, 16, 16)           # K-tiles per layer (C = 1024, 2048, 2048)

# DVE/Pool split of z-formation k-tiles per layer (DVE takes [0:d), Pool rest)
DVE_TILES = (6, 12, 13)

_CACHE = {}


def _build_module():
    import concourse.bass as bass
    import concourse.bacc as bacc
    import concourse.tile as tile
    from concourse import mybir
    from contextlib import ExitStack

    f16 = mybir.dt.float16
    f32 = mybir.dt.float32
    Relu = mybir.ActivationFunctionType.Relu
    Alu = mybir.AluOpType

    nc = bacc.Bacc(None, target_bir_lowering=False)

    xT = nc.dram_tensor("xT", [F, NTOT], f16, kind="ExternalInput")
    wdr = [
        nc.dram_tensor("w0", [128, TS[0] * O], f16, kind="ExternalInput"),
        nc.dram_tensor("w1", [128, TS[1] * O], f16, kind="ExternalInput"),
        nc.dram_tensor("w2", [128, TS[2] * O], f16, kind="ExternalInput"),
    ]
    bias = nc.dram_tensor("bias", [O, 3], f32, kind="ExternalInput")
    out = nc.dram_tensor("out", [2 * O, BC], f32, kind="ExternalOutput")

    XS = NTOT  # xT row stride (elements)

    with tile.TileContext(nc, pool_alloc_mode="queue") as tc, ExitStack() as ctx:
        singles = ctx.enter_context(tc.tile_pool(name="singles", bufs=1))
        xr0p = ctx.enter_context(tc.tile_pool(name="xr0p", bufs=4))
        xr12p = ctx.enter_context(tc.tile_pool(name="xr12p", bufs=5))
        ztp = ctx.enter_context(tc.tile_pool(name="ztp", bufs=3))
        hp = ctx.enter_context(tc.tile_pool(name="hp", bufs=3))
        kp = ctx.enter_context(tc.tile_pool(name="kp", bufs=2))
        ps = ctx.enter_context(tc.tile_pool(name="ps", bufs=6, space="PSUM"))

        # ---- preload (all on SP/HWDGE) ----
        wts = []
        for i, T in enumerate(TS):
            wt = singles.tile([128, T, O], f16, tag=f"w{i}")
            nc.sync.dma_start(out=wt[:], in_=wdr[i][:])
            wts.append(wt)
        bt = singles.tile([O, 3], f32)
        nc.sync.dma_start(out=bt[:], in_=bias[:])
        acc0 = singles.tile([128, BC], f32, tag="acc0")
        acc1 = singles.tile([128, BC], f32, tag="acc1")

        # per-chunk state carried between stages
        xr0_t = [None] * NCHUNKS
        xr12_t = [None] * NCHUNKS
        zt_t = [None] * NCHUNKS
        h_t = [None, None]      # h_t[i] for layer i+1 input, indexed by chunk
        h1_t = [None] * NCHUNKS
        h2_t = [None] * NCHUNKS
        ps_t = [None] * NCHUNKS

        def dma_xr0(c):
            xr0 = xr0p.tile([128, TS[0] + 1, NCHUNK], f16, tag="xr0")
            for q in range(4):
                nc.sync.dma_start(
                    out=xr0[32 * q:32 * (q + 1), 0:TS[0], :],
                    in_=bass.AP(tensor=xT[:].tensor, offset=q * XS + c * NCHUNK,
                                ap=[[0, 32], [4 * XS, TS[0]], [1, NCHUNK]]))
            # column TS[0]: xq[p, j] = x[p % 32, n0 + j] (layer-0 g-factor)
            nc.sync.dma_start(
                out=xr0[:, TS[0], :],
                in_=bass.AP(tensor=xT[:].tensor, offset=c * NCHUNK,
                            ap=[[0, 4], [XS, 32], [1, NCHUNK]]))
            xr0_t[c] = xr0

        def dma_xr12(c):
            xr12 = xr12p.tile([128, TS[1], NCHUNK], f16, tag="xr12")
            for s2 in range(2):
                nc.sync.dma_start(
                    out=xr12[64 * s2:64 * (s2 + 1), :, :],
                    in_=bass.AP(tensor=xT[:].tensor, offset=s2 * XS + c * NCHUNK,
                                ap=[[0, 64], [2 * XS, TS[1]], [1, NCHUNK]]))
            xr12_t[c] = xr12

        def zmul(dst, src, hsrc, hoff, i):
            T, d = TS[i], DVE_TILES[i]
            for eng, lo, hi in ((nc.vector, 0, d), (nc.gpsimd, d, T)):
                hb = bass.AP(tensor=hsrc.tensor, offset=hoff,
                             ap=[hsrc.ap[0], [0, hi - lo], [1, NCHUNK]])
                eng.tensor_mul(dst[:, lo:hi, :], src[:, lo:hi, :], hb)

        def z0(c):
            xr0 = xr0_t[c]
            zmul(xr0, xr0, xr0, xr0.offset + TS[0] * NCHUNK, 0)

        def z1(c):
            zt = ztp.tile([128, TS[1], NCHUNK], f16, tag="zt")
            zmul(zt, xr12_t[c], h1_t[c], h1_t[c].offset, 1)
            zt_t[c] = zt

        def z2(c):
            zmul(xr12_t[c], xr12_t[c], h2_t[c], h2_t[c].offset, 2)

        def mm(c, i, zt):
            psum = ps.tile([128, NCHUNK], f32)
            T = TS[i]
            for t in range(T):
                nc.tensor.matmul(psum[:], wts[i][:, t, :], zt[:, t, :],
                                 start=(t == 0), stop=(t == T - 1))
            ps_t[c] = psum

        def evac_keep_h(c, i, h_list):
            psum = ps_t[c]
            hbuf = hp.tile([128, NCHUNK], f16, tag=f"h{i}")
            nc.scalar.activation(out=hbuf[64:128, :], in_=psum[64:128, :],
                                 func=Relu, bias=bt[64:128, i:i + 1], scale=1.0)
            nc.scalar.activation(out=hbuf[0:64, :], in_=psum[64:128, :],
                                 func=Relu, bias=bt[64:128, i:i + 1], scale=1.0)
            kt = kp.tile([64, NCHUNK], f32, tag=f"k{i}")
            nc.scalar.activation(out=kt[:], in_=psum[0:64, :],
                                 func=Relu, bias=bt[0:64, i:i + 1], scale=1.0)
            h_list[c] = hbuf
            return kt

        def evac_full(c):
            psum = ps_t[c]
            kt = kp.tile([128, NCHUNK], f32, tag="k2")
            nc.scalar.activation(out=kt[:], in_=psum[:],
                                 func=Relu, bias=bt[:, 2:3], scale=1.0)
            return kt

        def esum(kt, acc, row0, np_, c):
            nc.vector.tensor_reduce(
                acc[row0:row0 + np_, c * BCH:(c + 1) * BCH],
                kt[:].rearrange("p (b e) -> p b e", e=E),
                axis=mybir.AxisListType.X, op=Alu.add)

        # ---- prologue ----
        dma_xr0(0)
        dma_xr12(0)
        dma_xr0(1)
        z0(0)

        # ---- pipelined stages ----
        for s in range(NCHUNKS + 2):
            if s + 1 < NCHUNKS:
                dma_xr12(s + 1)
            if s + 2 < NCHUNKS:
                dma_xr0(s + 2)
            if s < NCHUNKS:
                mm(s, 0, xr0_t[s])
                kt0 = evac_keep_h(s, 0, h1_t)
            if s + 1 < NCHUNKS:
                z0(s + 1)
            if s < NCHUNKS:
                z1(s)
                esum(kt0, acc0, 0, 64, s)
            c1 = s - 1
            if 0 <= c1 < NCHUNKS:
                mm(c1, 1, zt_t[c1])
                kt1 = evac_keep_h(c1, 1, h2_t)
                z2(c1)
                esum(kt1, acc0, 64, 64, c1)
            c2 = s - 2
            if 0 <= c2 < NCHUNKS:
                mm(c2, 2, xr12_t[c2])
                kt2 = evac_full(c2)
                esum(kt2, acc1, 0, 128, c2)

        # ---- output ----
        nc.sync.dma_start(out=out[0:128, :], in_=acc0[:])
        nc.sync.dma_start(out=out[128:256, :], in_=acc1[:])

    nc.compile()
    return nc


def _get_nc():
    if "nc" not in _CACHE:
        _CACHE["nc"] = _build_module()
    return _CACHE["nc"]


def _prep_inputs(x, W0, b0, W1, b1, W2, b2):
    """Host-side prep: shard batch, transpose/convert. Returns in_maps."""
    x = np.asarray(x, dtype=np.float32)
    Ws = [np.asarray(W, dtype=np.float32) for W in (W0, W1, W2)]
    bs = [np.asarray(b, dtype=np.float32) for b in (b0, b1, b2)]

    # wdr[i][k, t*O + m] = W_i[m, 128t + k]
    wts = []
    for W, T in zip(Ws, TS):
        wt = W.T.reshape(T, 128, O).transpose(1, 0, 2).reshape(128, T * O)
        wts.append(np.ascontiguousarray(wt).astype(np.float16))
    bias = np.stack(bs, axis=1).astype(np.float32)  # (128, 3)

    in_maps = []
    for core in range(N_CORES):
        xc = x[core * BC:(core + 1) * BC]  # (512, 32, 16)
        xTc = np.ascontiguousarray(
            xc.transpose(1, 0, 2).reshape(F, NTOT)).astype(np.float16)
        in_maps.append({
            "xT": xTc,
            "w0": wts[0], "w1": wts[1], "w2": wts[2],
            "bias": bias,
        })
    return in_maps


def kernel(x, W0, b0, W1, b1, W2, b2, _trace=False):
    from concourse.bass_utils import run_bass_kernel_spmd

    nc = _get_nc()
    in_maps = _prep_inputs(x, W0, b0, W1, b1, W2, b2)
    res = run_bass_kernel_spmd(nc, in_maps, core_ids=list(range(N_CORES)),
                               trace=_trace)
    outs = [res.results[i]["out"] for i in range(N_CORES)]  # each (256, 512)
    full = np.concatenate(outs, axis=1)                     # (256, 4096)
    result = np.ascontiguousarray(full.T).astype(np.float32)
    if _trace:
        return result, res
    return result
